# revision 1
# baseline (speedup 1.0000x reference)
"""Distributed grid self-attention (AlphaFold triangle-attention style) on 8 trn2 cores.

Layout strategy (per core, 48 grid rows):
  - LayerNorm'd pair kept transposed in SBUF: xT [c=128, tokens].
  - Per attention row b: logits computed TRANSPOSED (logits.T [j, i]) so that
    softmax's masked sum-over-j and the AV contraction both have j on partitions.
  - Pair bias [H,N,N]: each core computes its partial [H, 48*384] from its own
    LN'd rows, AllGather -> full bias, PE-transposed to [j, i] tiles.
  - Mask enters as the per-partition bias operand of the ACT exp (free).
  - Softmax denominator: col-tiled matmuls with a constant-2.0 stationary operand
    replicate each head's sum over its 32 partitions, so one reciprocal op also
    performs the broadcast needed for normalization.
"""

import sys

for _p in ("/opt/trn_rl_repo",):
    if _p not in sys.path:
        sys.path.insert(0, _p)

import numpy as np

import concourse.bass as bass
import concourse.mybir as mybir
import concourse.tile as tile
from concourse import bacc
from concourse.bass_utils import run_bass_kernel_spmd

F32 = mybir.dt.float32
BF16 = mybir.dt.bfloat16
I32 = mybir.dt.int32
AF = mybir.ActivationFunctionType
OP = mybir.AluOpType

N, C, H, D = 384, 128, 4, 32
NCORES = 8
R = N // NCORES          # 48 rows per core
T = R * N                # 18432 tokens per core
NT = T // 128            # 144 tiles of 128 tokens
CHUNK = 48               # LN tiles per stats batch (16 attention rows)
INVSQ_D = float(1.0 / np.sqrt(D))
RSQRT_MAGIC = 0x5F3759DF


def build_program():
    nc = bacc.Bacc(
        "TRN2",
        target_bir_lowering=False,
        debug=False,
        enable_asserts=False,
        num_devices=NCORES,
    )
    f = F32
    # ---- external IO (per core) ----
    pair_s = nc.dram_tensor("pair_s", [T, C], f, kind="ExternalInput").ap()
    mask_s = nc.dram_tensor("mask_s", [R, N], f, kind="ExternalInput").ap()
    wq_d = nc.dram_tensor("wq", [C, C], f, kind="ExternalInput").ap()
    wk_d = nc.dram_tensor("wk", [C, C], f, kind="ExternalInput").ap()
    wv_d = nc.dram_tensor("wv", [C, C], f, kind="ExternalInput").ap()
    wg_d = nc.dram_tensor("wg", [C, C], f, kind="ExternalInput").ap()
    wo_d = nc.dram_tensor("wo", [C, C], BF16, kind="ExternalInput").ap()
    identb_d = nc.dram_tensor("identb", [C, C], BF16, kind="ExternalInput").ap()
    twosb_d = nc.dram_tensor("twosb", [C, D], BF16, kind="ExternalInput").ap()
    bwT_d = nc.dram_tensor("bwT", [C, 32], f, kind="ExternalInput").ap()
    qb_d = nc.dram_tensor("qb", [C, 1], f, kind="ExternalInput").ap()
    kb_d = nc.dram_tensor("kb", [C, 1], f, kind="ExternalInput").ap()
    gbh_d = nc.dram_tensor("gbh", [C, 1], f, kind="ExternalInput").ap()
    vbrow_d = nc.dram_tensor("vbrow", [1, C], f, kind="ExternalInput").ap()
    bb128_d = nc.dram_tensor("bb128", [C, 1], f, kind="ExternalInput").ap()
    ident_d = nc.dram_tensor("ident", [C, C], f, kind="ExternalInput").ap()
    out_s = nc.dram_tensor("out_s", [T, C], f, kind="ExternalOutput").ap()
    # ---- internal DRAM for the collective ----
    pb_dram = nc.dram_tensor("pb_dram", [H, T], f)
    gat_dram = nc.dram_tensor("gat_dram", [NCORES, H, R, N], f, addr_space="Shared")

    with tile.TileContext(nc) as tc:
        _emit(tc, locals())
    nc.compile()
    return nc


def _chain(insts):
    from concourse.tile_rust import add_dep_helper
    for a, b in zip(insts[1:], insts[:-1]):
        add_dep_helper(a.ins, b.ins, sync=False, reason="same-bank psum group order")


def _emit(tc, io):
    nc = tc.nc
    f = F32
    pair_s, mask_s, out_s = io["pair_s"], io["mask_s"], io["out_s"]
    pb_dram, gat_dram = io["pb_dram"], io["gat_dram"]

    from contextlib import ExitStack

    with ExitStack() as ctx:
        const = ctx.enter_context(tc.tile_pool(name="const", bufs=1))

        # ---- constants / weights ----
        wq = const.tile([C, C], f, tag="wq")
        wk = const.tile([C, C], f, tag="wk")
        wv = const.tile([C, C], f, tag="wv")
        wg = const.tile([C, C], f, tag="wg")
        wo = const.tile([C, C], BF16, tag="wo")
        identb = const.tile([C, C], BF16, tag="identb")
        twosb = const.tile([C, D], BF16, tag="twosb")
        ident = const.tile([C, C], f, tag="ident")
        bwT = const.tile([C, 32], f, tag="bwT")
        qb = const.tile([C, 1], f, tag="qb")
        kb = const.tile([C, 1], f, tag="kb")
        gbh = const.tile([C, 1], f, tag="gbh")
        vbrow = const.tile([1, C], f, tag="vbrow")
        bb128 = const.tile([C, 1], f, tag="bb128")
        for t_, d_ in (
            (wq, io["wq_d"]), (wk, io["wk_d"]), (wv, io["wv_d"]),
            (wg, io["wg_d"]), (wo, io["wo_d"]), (ident, io["ident_d"]),
            (bwT, io["bwT_d"]), (qb, io["qb_d"]), (kb, io["kb_d"]),
            (identb, io["identb_d"]), (twosb, io["twosb_d"]),
            (gbh, io["gbh_d"]), (vbrow, io["vbrow_d"]), (bb128, io["bb128_d"]),
        ):
            nc.sync.dma_start(t_[:], d_)

        ones1 = const.tile([1, C], f, tag="ones1")
        nc.gpsimd.memset(ones1[:], 1.0)

        # mask bias columns: mb[jp, 3*r + jc] = 1e9*(mask[r, jc*128+jp] - 1)
        mbraw = const.tile([128, 3 * R], f, tag="mbraw")
        nc.sync.dma_start(mbraw[:], mask_s.rearrange("r (c p) -> p (r c)", p=128))
        mbb = const.tile([128, 3 * R], f, tag="mbb")
        nc.vector.tensor_scalar(mbb[:], mbraw[:], 1e9, -1e9, OP.mult, OP.add)

        # persistent LN'd transposed pair
        xT = const.tile([128, T], f, tag="xT")
        # persistent bias (transposed [j, i]) tiles: [jp, (h, jc), i]
        biasT = const.tile([128, H * 3, N], BF16, tag="biasT")

        # ================= prologue: LN + partial bias =================
        with ExitStack() as pctx:
            pairp = pctx.enter_context(tc.tile_pool(name="pair", bufs=2 * CHUNK // 4))
            statsp = pctx.enter_context(tc.tile_pool(name="stats", bufs=2))
            scr = pctx.enter_context(tc.tile_pool(name="scr", bufs=10))
            xhp = pctx.enter_context(tc.tile_pool(name="xh", bufs=6))
            tpsum = pctx.enter_context(tc.tile_pool(name="tpsum", bufs=2, space="PSUM"))
            pbps = pctx.enter_context(tc.tile_pool(name="pbps", bufs=2, space="PSUM"))
            pbsb = pctx.enter_context(tc.tile_pool(name="pbsb", bufs=2))

            for cc in range(T // (128 * CHUNK)):  # 3 chunks of 48 tiles
                p4s = []
                stats = statsp.tile([128, CHUNK, 6], f, tag="st")
                for k4 in range(CHUNK // 4):
                    kk = cc * CHUNK + k4 * 4
                    p4 = pairp.tile([128, 4, C], f, tag="p4")
                    nc.sync.dma_start(
                        p4[:], pair_s[kk * 128:(kk + 4) * 128, :].rearrange(
                            "(t p) c -> p t c", p=128
                        ),
                    )
                    for s in range(4):
                        nc.vector.bn_stats(stats[:, k4 * 4 + s, :], p4[:, s, :])
                    p4s.append(p4)
                # combine: mean, var
                me, mo = stats[:, :, 1], stats[:, :, 4]
                m2e, m2o = stats[:, :, 2], stats[:, :, 5]
                d_ = scr.tile([128, CHUNK], f, tag="a")
                d2 = scr.tile([128, CHUNK], f, tag="b")
                t2 = scr.tile([128, CHUNK], f, tag="c")
                veps = scr.tile([128, CHUNK], f, tag="d")
                mu = scr.tile([128, CHUNK], f, tag="e")
                nc.vector.tensor_sub(d_[:], me, mo)
                nc.vector.tensor_mul(d2[:], d_[:], d_[:])
                nc.vector.tensor_add(t2[:], m2e, m2o)
                # var*128 = t2 + 32*d2 ; veps = var + 1e-5
                nc.vector.scalar_tensor_tensor(veps[:], d2[:], 32.0, t2[:], OP.mult, OP.add)
                nc.vector.tensor_scalar(veps[:], veps[:], 1.0 / 128.0, 1e-5, OP.mult, OP.add)
                nc.vector.tensor_add(mu[:], me, mo)
                nc.vector.tensor_scalar_mul(mu[:], mu[:], 0.5)
                # rstd = rsqrt(veps): magic seed + 3 Newton iterations
                magic = scr.tile([128, CHUNK], I32, tag="f")
                nc.vector.memset(magic[:], RSQRT_MAGIC)
                sh = scr.tile([128, CHUNK], I32, tag="g")
                nc.vector.tensor_scalar(sh[:], veps[:].bitcast(I32), 1, None, OP.logical_shift_right)
                y = scr.tile([128, CHUNK], f, tag="h")
                nc.vector.tensor_tensor(y[:].bitcast(I32), magic[:], sh[:], OP.subtract)
                a_ = scr.tile([128, CHUNK], f, tag="i")
                for _ in range(3):
                    nc.vector.tensor_mul(a_[:], y[:], y[:])
                    nc.vector.tensor_mul(a_[:], a_[:], veps[:])
                    nc.vector.tensor_scalar(a_[:], a_[:], -0.5, 1.5, OP.mult, OP.add)
                    nc.vector.tensor_mul(y[:], y[:], a_[:])
                # apply + transpose each tile
                for k in range(CHUNK):
                    kk = cc * CHUNK + k
                    xh = xhp.tile([128, C], f, tag="xh")
                    nc.vector.tensor_scalar(
                        xh[:], p4s[k // 4][:, k % 4, :], mu[:, k:k + 1], y[:, k:k + 1],
                        OP.subtract, OP.mult,
                    )
                    pst = tpsum.tile([128, 128], f, tag="tp")
                    nc.tensor.transpose(pst[:], xh[:], ident[:])
                    if k % 2 == 0:
                        nc.vector.tensor_copy(xT[:, kk * 128:(kk + 1) * 128], pst[:])
                    else:
                        nc.scalar.copy(xT[:, kk * 128:(kk + 1) * 128], pst[:])
                # partial bias for this chunk: out[32s+h, tok] over 4 col-tiles
                t0 = cc * CHUNK * 128
                for g3 in range(3):
                    ps = pbps.tile([128, 512], f, tag="pb")
                    mms = []
                    for s in range(4):
                        tk = t0 + g3 * 2048 + s * 512
                        mms.append(nc.tensor.matmul(
                            ps[32 * s:32 * s + 32, :],
                            lhsT=bwT[:, :],
                            rhs=xT[:, tk:tk + 512],
                            start=True, stop=True,
                            tile_position=(0, 32 * s),
                        ))
                    _chain(mms)
                    pbt = pbsb.tile([128, 512], f, tag="pbt")
                    nc.vector.tensor_scalar(pbt[:], ps[:], bb128[:, :], None, OP.add)
                    for s in range(4):
                        tk = t0 + g3 * 2048 + s * 512
                        nc.sync.dma_start(
                            pb_dram[:, tk:tk + 512], pbt[32 * s:32 * s + 4, :]
                        )

        # ================= collective =================
        nc.gpsimd.collective_compute(
            "AllGather",
            OP.bypass,
            replica_groups=[list(range(NCORES))],
            ins=[pb_dram[:].opt()],
            outs=[gat_dram[:].opt()],
        )

        # ================= main loop over attention rows =================
        mmps = ctx.enter_context(tc.tile_pool(name="mmps", bufs=2, space="PSUM"))
        Lps = ctx.enter_context(tc.tile_pool(name="Lps", bufs=1, space="PSUM"))
        ops_ = ctx.enter_context(tc.tile_pool(name="ops", bufs=1, space="PSUM"))
        sps = ctx.enter_context(tc.tile_pool(name="sps", bufs=1, space="PSUM"))
        rowp = ctx.enter_context(tc.tile_pool(name="row", bufs=2))
        projp = ctx.enter_context(tc.tile_pool(name="projp", bufs=13))
        ep = ctx.enter_context(tc.tile_pool(name="ep", bufs=2))

        out_sb = None

        def emit_proj(r):
            xTr = xT[:, r * N:(r + 1) * N]
            # projections q, k, gate
            psq = mmps.tile([128, 512], f, tag="mm")
            nc.tensor.matmul(psq[:, :N], lhsT=wq[:], rhs=xTr, start=True, stop=True)
            q_sb = projp.tile([128, N], BF16, tag="q")
            nc.vector.tensor_scalar(q_sb[:], psq[:, :N], qb[:, :], INVSQ_D, OP.add, OP.mult)
            psk = mmps.tile([128, 512], f, tag="mm")
            nc.tensor.matmul(psk[:, :N], lhsT=wk[:], rhs=xTr, start=True, stop=True)
            k_sb = projp.tile([128, N], BF16, tag="k")
            nc.vector.tensor_scalar(k_sb[:], psk[:, :N], kb[:, :], None, OP.add)
            psg = mmps.tile([128, 512], f, tag="mm")
            nc.tensor.matmul(psg[:, :N], lhsT=wg[:], rhs=xTr, start=True, stop=True)
            g_sb = projp.tile([128, N], f, tag="g")
            nc.scalar.activation(g_sb[:], psg[:, :N], AF.Tanh, bias=gbh[:, :], scale=0.5)
            # v, computed directly transposed: vt[j, c] per jc block
            psv = mmps.tile([128, 512], f, tag="mm")
            vmms = []
            for jc in range(3):
                sl = psv[:, jc * 128:(jc + 1) * 128]
                vmms.append(nc.tensor.matmul(
                    sl, lhsT=ones1[:], rhs=vbrow[:], start=True, stop=False))
                vmms.append(nc.tensor.matmul(
                    sl, lhsT=xTr[:, jc * 128:(jc + 1) * 128], rhs=wv[:],
                    start=False, stop=True,
                ))
            _chain(vmms)
            vt_sb = projp.tile([128, 3 * C], BF16, tag="vt")
            nc.vector.tensor_copy(vt_sb[:], psv[:, :3 * C])
            return dict(q_sb=q_sb, k_sb=k_sb, g_sb=g_sb, vt_sb=vt_sb)

        def emit_rest(r, _t):
            nonlocal out_sb
            q_sb, k_sb = _t["q_sb"], _t["k_sb"]
            g_sb, vt_sb = _t["g_sb"], _t["vt_sb"]

            pso = ops_.tile([128, 512], f, tag="o")
            pss = sps.tile([128, 512], f, tag="s")
            e_t = ep.tile([128, 3, H, N], BF16, tag="E")
            omms, smms = [], []
            for jc in range(3):
                Lbig = Lps.tile([128, H, 512], f, tag="L")
                for h in range(H):
                    sl = Lbig[:, h, :N]
                    nc.tensor.matmul(
                        sl, lhsT=identb[:], rhs=biasT[:, h * 3 + jc, :],
                        start=True, stop=False,
                    )
                    nc.tensor.matmul(
                        sl,
                        lhsT=k_sb[32 * h:32 * h + 32, jc * 128:(jc + 1) * 128],
                        rhs=q_sb[32 * h:32 * h + 32, :],
                        start=False, stop=True,
                        tile_position=(32 * h, 0),
                    )
                nc.scalar.activation(
                    e_t[:, jc, :, :], Lbig[:, :, :N], AF.Exp,
                    bias=mbb[:, 3 * r + jc:3 * r + jc + 1], scale=1.0,
                )
                for h in range(H):
                    e_ap = e_t[:, jc, h, :]
                    omms.append(nc.tensor.matmul(
                        pso[32 * h:32 * h + 32, :N],
                        lhsT=vt_sb[:, jc * 128 + 32 * h:jc * 128 + 32 * h + 32],
                        rhs=e_ap,
                        start=(jc == 0), stop=(jc == 2),
                        tile_position=(0, 32 * h),
                        skip_group_check=True,
                    ))
                    smms.append(nc.tensor.matmul(
                        pss[32 * h:32 * h + 32, :N],
                        lhsT=twosb[:, :],
                        rhs=e_ap,
                        start=(jc == 0), stop=(jc == 2),
                        tile_position=(0, 32 * h),
                        skip_group_check=True,
                    ))
            _chain(omms)
            _chain(smms)
            # normalize + gate:  og = pso * (1 + tanh) * recip(2*sum)
            recip = rowp.tile([128, N], f, tag="rc")
            nc.vector.reciprocal_approx_fast(recip[:], pss[:, :N])
            gr = rowp.tile([128, N], f, tag="gr")
            nc.vector.scalar_tensor_tensor(gr[:], g_sb[:], 1.0, recip[:], OP.add, OP.mult)
            og = rowp.tile([128, N], BF16, tag="og")
            nc.vector.tensor_mul(og[:], pso[:, :N], gr[:])
            # output projection
            psf = mmps.tile([128, 512], f, tag="mm")
            fmms = []
            for ic in range(3):
                fmms.append(nc.tensor.matmul(
                    psf[:, ic * 128:(ic + 1) * 128],
                    lhsT=og[:, ic * 128:(ic + 1) * 128], rhs=wo[:],
                    start=True, stop=True,
                ))
            _chain(fmms)
            out_sb = rowp.tile([128, 3 * C], f, tag="out")
            nc.scalar.copy(out_sb[:], psf[:, :3 * C])
            nc.sync.dma_start(
                out_s[r * N:(r + 1) * N, :].rearrange("(c p) d -> p c d", p=128),
                out_sb[:],
            )

        PREF = 12
        pref = {r: emit_proj(r) for r in range(PREF)}

        # ================= bias gather -> [i, j] tiles -> transpose =================
        with ExitStack() as bctx:
            bijp = bctx.enter_context(tc.tile_pool(name="bij", bufs=1))
            bias_ij = bijp.tile([128, H * 3, N], f, tag="bij")  # [ip, (h, ic), j]
            for c8 in range(NCORES):
                lo, hi = 48 * c8, 48 * (c8 + 1)
                # split on 128-row tile boundaries
                cuts = sorted({lo, hi, *[b for b in (128, 256) if lo < b < hi]})
                for a0, a1 in zip(cuts[:-1], cuts[1:]):
                    ic = a0 // 128
                    nc.sync.dma_start(
                        bias_ij[a0 - 128 * ic:a1 - 128 * ic, ic::3, :],
                        gat_dram[c8, :, a0 - lo:a1 - lo, :].rearrange(
                            "h i j -> i h j"
                        ),
                    )
            for h in range(H):
                for jc in range(3):
                    pbt2 = mmps.tile([128, 512], f, tag="mm")
                    tps = []
                    for ic in range(3):
                        tps.append(nc.tensor.transpose(
                            pbt2[:, ic * 128:(ic + 1) * 128],
                            bias_ij[:, h * 3 + ic, jc * 128:(jc + 1) * 128],
                            ident[:],
                        ))
                    _chain(tps)
                    if (h * 3 + jc) % 2 == 0:
                        nc.vector.tensor_copy(biasT[:, h * 3 + jc, :], pbt2[:, :N])
                    else:
                        nc.scalar.copy(biasT[:, h * 3 + jc, :], pbt2[:, :N])

        for r in range(R):
            t = pref.pop(r) if r in pref else emit_proj(r)
            emit_rest(r, t)

_NC = None


def _get_nc():
    global _NC
    if _NC is None:
        _NC = build_program()
    return _NC


def make_in_maps(pair, mask, ln_w, ln_b, bias_w, q_w, k_w, v_w, g_w, o_w):
    pair = np.asarray(pair, np.float32)
    mask = np.asarray(mask, np.float32)
    ln_w = np.asarray(ln_w, np.float32)
    ln_b = np.asarray(ln_b, np.float32)
    bias_w = np.asarray(bias_w, np.float32)

    def fold(w):
        w = np.asarray(w, np.float32)
        wm = w * ln_w[None, :]
        return np.ascontiguousarray(wm.T), np.ascontiguousarray(
            (w @ ln_b).reshape(C, 1)
        )

    wq, qb = fold(q_w)
    wk, kb = fold(k_w)
    wv, vb = fold(v_w)
    wg, gb = fold(g_w)
    import ml_dtypes
    wo = np.ascontiguousarray(np.asarray(o_w, np.float32).T).astype(ml_dtypes.bfloat16)
    bwm = bias_w * ln_w[None, :]
    bwT = np.zeros((C, 32), np.float32)                    # [C, 32], heads in cols 0..4
    bwT[:, :H] = bwm.T
    bb = (bias_w @ ln_b).astype(np.float32)                # [H]
    bb128 = np.zeros((C, 1), np.float32)
    for s in range(4):
        bb128[32 * s:32 * s + 4, 0] = bb
    common = {
        "wq": wq, "wk": wk, "wv": wv, "wg": wg, "wo": wo,
        "bwT": bwT,
        "qb": qb, "kb": kb,
        "gbh": np.ascontiguousarray(0.5 * gb),
        "identb": np.eye(C, dtype=ml_dtypes.bfloat16),
        "twosb": np.full((C, D), 2.0, dtype=ml_dtypes.bfloat16),
        "vbrow": np.ascontiguousarray(vb.reshape(1, C)),
        "bb128": bb128,
        "ident": np.eye(C, dtype=np.float32),
    }
    in_maps = []
    for c in range(NCORES):
        m = dict(common)
        m["pair_s"] = np.ascontiguousarray(
            pair[c * R:(c + 1) * R].reshape(T, C)
        )
        m["mask_s"] = np.ascontiguousarray(mask[c * R:(c + 1) * R])
        in_maps.append(m)
    return in_maps


def kernel(**inputs) -> np.ndarray:
    nc = _get_nc()
    in_maps = make_in_maps(**inputs)
    res = run_bass_kernel_spmd(nc, in_maps, core_ids=list(range(NCORES)))
    shards = [res.results[c]["out_s"].reshape(R, N, C) for c in range(NCORES)]
    return np.ascontiguousarray(np.concatenate(shards, axis=0))



# revision 7
# speedup vs baseline: 1.6967x; 1.6967x over previous
"""Distributed grid self-attention (AlphaFold triangle-attention style) on 8 trn2 cores.

v2 layout strategy (per core, 48 grid rows):
  - Merged prologue: LN stats + apply + PE-transpose + ALL projections
    (q, k, v, gate) chunk-by-chunk, so xT only lives per-chunk and the PE
    stays busy from the start.  q/k stored c-major bf16 [128, T]; v stored
    token-major bf16 tiles [128, 144, 128] (AV stationary directly); gate
    stored as tanh(g/2) bf16 c-major.
  - Pair bias handled MULTIPLICATIVELY: partial bias (bf16) -> AllGather ->
    transpose -> E = exp(biasT) [j, i] bf16.  Per-row softmax numerator is
    w = exp(q.k + maskbias) * E, so no PE identity-matmuls are needed to
    inject the bias into PSUM.
  - Main loop pipelined at (jc, head-pair) unit granularity with a stagger
    of 3 units between the kq matmuls and the AV/sum matmuls, so the PE
    never waits on ACT exp or the DVE multiply.
  - PSUM budget: Lbig4 [128,4,512] static (2 half buffers) + pso + pss +
    psf = 7 banks.
"""

import sys

for _p in ("/opt/trn_rl_repo",):
    if _p not in sys.path:
        sys.path.insert(0, _p)

import numpy as np

import concourse.bass as bass
import concourse.mybir as mybir
import concourse.tile as tile
from concourse import bacc
from concourse.bass_utils import run_bass_kernel_spmd

F32 = mybir.dt.float32
BF16 = mybir.dt.bfloat16
I32 = mybir.dt.int32
AF = mybir.ActivationFunctionType
OP = mybir.AluOpType

N, C, H, D = 384, 128, 4, 32
NCORES = 8
R = N // NCORES          # 48 rows per core
T = R * N                # 18432 tokens per core
NT = T // 128            # 144 tiles of 128 tokens
CHUNK = 16               # LN/proj tiles per chunk
NCHUNK = NT // CHUNK     # 9 chunks
RSQRT_MAGIC = 0x5F3759DF
STAGGER = 3              # kq -> AV pipeline distance in units


def build_program(has_vb=False):
    nc = bacc.Bacc(
        "TRN2",
        target_bir_lowering=False,
        debug=False,
        enable_asserts=False,
        num_devices=NCORES,
    )
    f = F32
    # ---- external IO (per core) ----
    pair_s = nc.dram_tensor("pair_s", [T, C], f, kind="ExternalInput").ap()
    mask_s = nc.dram_tensor("mask_s", [R, N], f, kind="ExternalInput").ap()
    wq_d = nc.dram_tensor("wq", [C, C], BF16, kind="ExternalInput").ap()
    wk_d = nc.dram_tensor("wk", [C, C], BF16, kind="ExternalInput").ap()
    wv_d = nc.dram_tensor("wv", [C, C], BF16, kind="ExternalInput").ap()
    wg_d = nc.dram_tensor("wg", [C, C], BF16, kind="ExternalInput").ap()
    wo_d = nc.dram_tensor("wo", [C, C], BF16, kind="ExternalInput").ap()
    identb_d = nc.dram_tensor("identb", [C, C], BF16, kind="ExternalInput").ap()
    twosb_d = nc.dram_tensor("twosb", [C, D], BF16, kind="ExternalInput").ap()
    bwT_d = nc.dram_tensor("bwT", [C, 32], BF16, kind="ExternalInput").ap()
    qb_d = nc.dram_tensor("qb", [C, 1], f, kind="ExternalInput").ap()
    kb_d = nc.dram_tensor("kb", [C, 1], f, kind="ExternalInput").ap()
    gbh_d = nc.dram_tensor("gbh", [C, 1], f, kind="ExternalInput").ap()
    vb_d = nc.dram_tensor("vb", [C, 1], f, kind="ExternalInput").ap()
    bb128_d = nc.dram_tensor("bb128", [C, 1], f, kind="ExternalInput").ap()
    ident_d = nc.dram_tensor("ident", [C, C], f, kind="ExternalInput").ap()
    out_s = nc.dram_tensor("out_s", [T, C], f, kind="ExternalOutput").ap()
    # ---- internal DRAM for the collective ----
    pb_dram = nc.dram_tensor("pb_dram", [H, T], BF16)
    gat_dram = nc.dram_tensor("gat_dram", [NCORES, H, R, N], BF16, addr_space="Shared")

    with tile.TileContext(nc) as tc:
        _emit(tc, locals(), has_vb)
    nc.compile()
    return nc


def _chain(insts):
    from concourse.tile_rust import add_dep_helper
    for a, b in zip(insts[1:], insts[:-1]):
        add_dep_helper(a.ins, b.ins, sync=False, reason="same-bank psum group order")


def _emit(tc, io, has_vb):
    nc = tc.nc
    f = F32
    pair_s, mask_s, out_s = io["pair_s"], io["mask_s"], io["out_s"]
    pb_dram, gat_dram = io["pb_dram"], io["gat_dram"]

    from contextlib import ExitStack

    with ExitStack() as ctx:
        const = ctx.enter_context(tc.tile_pool(name="const", bufs=1))

        # ---- constants / weights ----
        wq = const.tile([C, C], BF16, tag="wq")
        wk = const.tile([C, C], BF16, tag="wk")
        wv = const.tile([C, C], BF16, tag="wv")
        wg = const.tile([C, C], BF16, tag="wg")
        wo = const.tile([C, C], BF16, tag="wo")
        identb = const.tile([C, C], BF16, tag="identb")
        twosb = const.tile([C, D], BF16, tag="twosb")
        ident = const.tile([C, C], f, tag="ident")
        bwT = const.tile([C, 32], BF16, tag="bwT")
        qb = const.tile([C, 1], f, tag="qb")
        kb = const.tile([C, 1], f, tag="kb")
        gbh = const.tile([C, 1], f, tag="gbh")
        vb = const.tile([C, 1], f, tag="vb")
        bb128 = const.tile([C, 1], f, tag="bb128")
        for t_, d_ in (
            (wq, io["wq_d"]), (wk, io["wk_d"]), (wv, io["wv_d"]),
            (wg, io["wg_d"]), (wo, io["wo_d"]), (ident, io["ident_d"]),
            (bwT, io["bwT_d"]), (qb, io["qb_d"]), (kb, io["kb_d"]),
            (identb, io["identb_d"]), (twosb, io["twosb_d"]),
            (gbh, io["gbh_d"]), (vb, io["vb_d"]), (bb128, io["bb128_d"]),
        ):
            nc.sync.dma_start(t_[:], d_)

        # mask bias columns: mb[jp, 3*r + jc] = 1e9*(mask[r, jc*128+jp] - 1)
        mbraw = const.tile([128, 3 * R], f, tag="mbraw")
        nc.sync.dma_start(mbraw[:], mask_s.rearrange("r (c p) -> p (r c)", p=128))
        mbb = const.tile([128, 3 * R], f, tag="mbb")
        nc.vector.tensor_scalar(mbb[:], mbraw[:], 1e9, -1e9, OP.mult, OP.add)

        # ---- persistent projected tensors ----
        q_all = const.tile([128, T], BF16, tag="q_all")
        k_all = const.tile([128, T], BF16, tag="k_all")
        v_tok = const.tile([128, NT, 128], BF16, tag="v_tok")
        g_all = const.tile([128, T], BF16, tag="g_all")
        # E factors per unit (jc, hp): exp of transposed pair bias
        E_t = const.tile([128, 6, 2, N], BF16, tag="E_t")

        # ================= prologue: LN + projections + partial bias =====
        with ExitStack() as pctx:
            pairp = pctx.enter_context(tc.tile_pool(name="pair", bufs=8))
            statsp = pctx.enter_context(tc.tile_pool(name="stats", bufs=2))
            scr = pctx.enter_context(tc.tile_pool(name="scr", bufs=10))
            xhp = pctx.enter_context(tc.tile_pool(name="xh", bufs=4))
            xTp = pctx.enter_context(tc.tile_pool(name="xTc", bufs=2))
            pbsb = pctx.enter_context(tc.tile_pool(name="pbsb", bufs=2))
            tps = pctx.enter_context(tc.tile_pool(name="tps", bufs=2, space="PSUM"))
            vps = pctx.enter_context(tc.tile_pool(name="vps", bufs=2, space="PSUM"))
            pps = pctx.enter_context(tc.tile_pool(name="pps", bufs=4, space="PSUM"))

            for cc in range(NCHUNK):
                t0 = cc * CHUNK * 128  # first token of chunk
                p4s = []
                stats = statsp.tile([128, CHUNK, 6], f, tag="st")
                for k4 in range(CHUNK // 4):
                    kk = cc * CHUNK + k4 * 4
                    p4 = pairp.tile([128, 4, C], f, tag="p4")
                    nc.sync.dma_start(
                        p4[:], pair_s[kk * 128:(kk + 4) * 128, :].rearrange(
                            "(t p) c -> p t c", p=128
                        ),
                    )
                    for s in range(4):
                        nc.vector.bn_stats(stats[:, k4 * 4 + s, :], p4[:, s, :])
                    p4s.append(p4)
                # combine: mean, var
                me, mo = stats[:, :, 1], stats[:, :, 4]
                m2e, m2o = stats[:, :, 2], stats[:, :, 5]
                d_ = scr.tile([128, CHUNK], f, tag="a")
                d2 = scr.tile([128, CHUNK], f, tag="b")
                t2 = scr.tile([128, CHUNK], f, tag="c")
                veps = scr.tile([128, CHUNK], f, tag="d")
                mu = scr.tile([128, CHUNK], f, tag="e")
                nc.vector.tensor_sub(d_[:], me, mo)
                nc.vector.tensor_mul(d2[:], d_[:], d_[:])
                nc.vector.tensor_add(t2[:], m2e, m2o)
                # var*128 = t2 + 32*d2 ; veps = var + 1e-5
                nc.vector.scalar_tensor_tensor(veps[:], d2[:], 32.0, t2[:], OP.mult, OP.add)
                nc.vector.tensor_scalar(veps[:], veps[:], 1.0 / 128.0, 1e-5, OP.mult, OP.add)
                nc.vector.tensor_add(mu[:], me, mo)
                nc.vector.tensor_scalar_mul(mu[:], mu[:], 0.5)
                # rstd = rsqrt(veps): magic seed + 3 Newton iterations
                magic = scr.tile([128, CHUNK], I32, tag="f")
                nc.vector.memset(magic[:], RSQRT_MAGIC)
                sh = scr.tile([128, CHUNK], I32, tag="g")
                nc.vector.tensor_scalar(sh[:], veps[:].bitcast(I32), 1, None, OP.logical_shift_right)
                y = scr.tile([128, CHUNK], f, tag="h")
                nc.vector.tensor_tensor(y[:].bitcast(I32), magic[:], sh[:], OP.subtract)
                a_ = scr.tile([128, CHUNK], f, tag="i")
                for _ in range(3):
                    nc.vector.tensor_mul(a_[:], y[:], y[:])
                    nc.vector.tensor_mul(a_[:], a_[:], veps[:])
                    nc.vector.tensor_scalar(a_[:], a_[:], -0.5, 1.5, OP.mult, OP.add)
                    nc.vector.tensor_mul(y[:], y[:], a_[:])
                # apply + transpose each tile into chunk-local xT (bf16)
                xTc = xTp.tile([128, CHUNK * 128], BF16, tag="xTc")
                for g4 in range(CHUNK // 4):
                    pst = tps.tile([128, 512], f, tag="tp")
                    tmms = []
                    for s in range(4):
                        k = g4 * 4 + s
                        xh = xhp.tile([128, C], f, tag="xh")
                        nc.vector.tensor_scalar(
                            xh[:], p4s[k // 4][:, k % 4, :], mu[:, k:k + 1], y[:, k:k + 1],
                            OP.subtract, OP.mult,
                        )
                        tmms.append(nc.tensor.transpose(
                            pst[:, s * 128:(s + 1) * 128], xh[:], ident[:]))
                    _chain(tmms)
                    if g4 % 2 == 0:
                        nc.vector.tensor_copy(xTc[:, g4 * 512:(g4 + 1) * 512], pst[:])
                    else:
                        nc.scalar.copy(xTc[:, g4 * 512:(g4 + 1) * 512], pst[:])
                # projections q, k (c-major bf16) and gate tanh
                for qtr in range(4):
                    sl = slice(qtr * 512, (qtr + 1) * 512)
                    gsl = slice(t0 + qtr * 512, t0 + (qtr + 1) * 512)
                    psq = pps.tile([128, 512], f, tag="pp")
                    nc.tensor.matmul(psq[:], lhsT=wq[:], rhs=xTc[:, sl], start=True, stop=True)
                    nc.scalar.activation(q_all[:, gsl], psq[:], AF.Identity, bias=qb[:, :], scale=1.0)
                    psk = pps.tile([128, 512], f, tag="pp")
                    nc.tensor.matmul(psk[:], lhsT=wk[:], rhs=xTc[:, sl], start=True, stop=True)
                    nc.scalar.activation(k_all[:, gsl], psk[:], AF.Identity, bias=kb[:, :], scale=1.0)
                    psg = pps.tile([128, 512], f, tag="pp")
                    nc.tensor.matmul(psg[:], lhsT=wg[:], rhs=xTc[:, sl], start=True, stop=True)
                    nc.scalar.activation(g_all[:, gsl], psg[:], AF.Tanh, bias=gbh[:, :], scale=0.5)
                # v token-major tiles
                for g4 in range(CHUNK // 4):
                    psv = vps.tile([128, 512], f, tag="vp")
                    vmms = []
                    for s in range(4):
                        k = g4 * 4 + s
                        vmms.append(nc.tensor.matmul(
                            psv[:, s * 128:(s + 1) * 128],
                            lhsT=xTc[:, k * 128:(k + 1) * 128], rhs=wv[:],
                            start=True, stop=True,
                        ))
                    _chain(vmms)
                    kk = cc * CHUNK + g4 * 4
                    nc.vector.tensor_copy(v_tok[:, kk:kk + 4, :], psv[:])
                # partial bias for this chunk: out[32s+h, tok] over col-tiles
                psb = pps.tile([128, 512], f, tag="pp")
                bmms = []
                for s in range(4):
                    tk = s * 512
                    bmms.append(nc.tensor.matmul(
                        psb[32 * s:32 * s + 32, :],
                        lhsT=bwT[:, :],
                        rhs=xTc[:, tk:tk + 512],
                        start=True, stop=True,
                        tile_position=(0, 32 * s),
                    ))
                _chain(bmms)
                pbt = pbsb.tile([128, 512], BF16, tag="pbt")
                nc.vector.tensor_scalar(pbt[:], psb[:], bb128[:, :], None, OP.add)
                for s in range(4):
                    tk = t0 + s * 512
                    nc.sync.dma_start(
                        pb_dram[:, tk:tk + 512], pbt[32 * s:32 * s + 4, :]
                    )

        # ================= collective =================
        nc.gpsimd.collective_compute(
            "AllGather",
            OP.bypass,
            replica_groups=[list(range(NCORES))],
            ins=[pb_dram[:].opt()],
            outs=[gat_dram[:].opt()],
        )

        # ================= bias gather -> transpose -> E = exp =========
        with ExitStack() as ectx:
            bijp = ectx.enter_context(tc.tile_pool(name="bij", bufs=1))
            eps = ectx.enter_context(tc.tile_pool(name="eps", bufs=2, space="PSUM"))
            bias_ij = bijp.tile([128, H * 3, N], BF16, tag="bij")  # [ip, (h, ic), j]
            for c8 in range(NCORES):
                lo, hi = R * c8, R * (c8 + 1)
                cuts = sorted({lo, hi, *[b for b in (128, 256) if lo < b < hi]})
                for a0, a1 in zip(cuts[:-1], cuts[1:]):
                    ic = a0 // 128
                    nc.sync.dma_start(
                        bias_ij[a0 - 128 * ic:a1 - 128 * ic, ic::3, :],
                        gat_dram[c8, :, a0 - lo:a1 - lo, :].rearrange(
                            "h i j -> i h j"
                        ),
                    )
            for h in range(H):
                for jc in range(3):
                    pst2 = eps.tile([128, 512], BF16, tag="ep")
                    tps2 = []
                    for ic in range(3):
                        tps2.append(nc.tensor.transpose(
                            pst2[:, ic * 128:(ic + 1) * 128],
                            bias_ij[:, h * 3 + ic, jc * 128:(jc + 1) * 128],
                            identb[:],
                        ))
                    _chain(tps2)
                    u = jc * 2 + h // 2
                    nc.scalar.activation(E_t[:, u, h % 2, :], pst2[:, :N], AF.Exp, scale=1.0)

        # ================= main loop over attention rows =================
        psump = ctx.enter_context(tc.tile_pool(name="mainps", bufs=1, space="PSUM"))
        Lbig4 = psump.tile([128, 4, 512], f, tag="L4")
        pso = psump.tile([128, 512], f, tag="pso")
        pss = psump.tile([128, 512], f, tag="pss")
        psf = psump.tile([128, 512], f, tag="psf")
        e0p = ctx.enter_context(tc.tile_pool(name="e0p", bufs=3))
        wp = ctx.enter_context(tc.tile_pool(name="wp", bufs=5))
        rowp = ctx.enter_context(tc.tile_pool(name="rowp", bufs=2))
        outp = ctx.enter_context(tc.tile_pool(name="outp", bufs=2))

        UNITS = R * 6
        wts = {}
        av_chain = {h: [] for h in range(H)}
        sm_chain = {h: [] for h in range(H)}
        kq_chain = []
        out_chain = []

        def emit_kq(t):
            r, u = divmod(t, 6)
            jc, hp = divmod(u, 2)
            Lh = Lbig4[:, 2 * hp:2 * hp + 2, :]
            for h2 in range(2):
                h = 2 * hp + h2
                mm = nc.tensor.matmul(
                    Lh[:, h2, :N],
                    lhsT=k_all[32 * h:32 * h + 32, r * N + jc * 128:r * N + (jc + 1) * 128],
                    rhs=q_all[32 * h:32 * h + 32, r * N:(r + 1) * N],
                    start=True, stop=True,
                    tile_position=(32 * h, 0),
                    skip_group_check=True,
                )
                kq_chain.append(mm)
            e0 = e0p.tile([128, 2, N], BF16, tag="e0")
            nc.scalar.activation(
                e0[:], Lh[:, :, :N], AF.Exp,
                bias=mbb[:, 3 * r + jc:3 * r + jc + 1], scale=1.0,
            )
            wt = wp.tile([128, 2, N], BF16, tag="w")
            if u % 2 == 0:
                nc.vector.tensor_mul(wt[:], e0[:], E_t[:, u, :, :])
            else:
                nc.gpsimd.tensor_mul(wt[:], e0[:], E_t[:, u, :, :])
            wts[t] = wt

        def emit_av(m):
            r, u = divmod(m, 6)
            jc, hp = divmod(u, 2)
            wt = wts.pop(m)
            for h2 in range(2):
                h = 2 * hp + h2
                av = nc.tensor.matmul(
                    pso[32 * h:32 * h + 32, :N],
                    lhsT=v_tok[:, 3 * r + jc, 32 * h:32 * h + 32],
                    rhs=wt[:, h2, :],
                    start=(jc == 0), stop=(jc == 2),
                    tile_position=(0, 32 * h),
                    skip_group_check=True,
                )
                av_chain[h].append(av)
                sm = nc.tensor.matmul(
                    pss[32 * h:32 * h + 32, :N],
                    lhsT=twosb[:, :],
                    rhs=wt[:, h2, :],
                    start=(jc == 0), stop=(jc == 2),
                    tile_position=(0, 32 * h),
                    skip_group_check=True,
                )
                sm_chain[h].append(sm)

        def emit_finish_dve(rf):
            # normalize + gate on DVE; must be emitted BEFORE av(rf+1, u0)
            rc = rowp.tile([128, N], f, tag="rc")
            nc.vector.reciprocal_approx_fast(rc[:], pss[:, :N])
            gr = rowp.tile([128, N], f, tag="gr")
            nc.vector.scalar_tensor_tensor(
                gr[:], g_all[:, rf * N:(rf + 1) * N], 1.0, rc[:], OP.add, OP.mult)
            og = rowp.tile([128, N], BF16, tag="og")
            if has_vb:
                ovb = rowp.tile([128, N], f, tag="ovb")
                nc.vector.scalar_tensor_tensor(
                    ovb[:], pss[:, :N], vb[:, :], pso[:, :N], OP.mult, OP.add)
                nc.vector.tensor_mul(og[:], ovb[:], gr[:])
            else:
                nc.vector.tensor_mul(og[:], pso[:, :N], gr[:])
            return og

        def emit_finish_pe(rf, og):
            fmms = []
            for ic in range(3):
                fmms.append(nc.tensor.matmul(
                    psf[:, ic * 128:(ic + 1) * 128],
                    lhsT=og[:, ic * 128:(ic + 1) * 128], rhs=wo[:],
                    start=True, stop=True,
                    skip_group_check=True,
                ))
            _chain(fmms)
            out_chain.extend(fmms)
            ob = outp.tile([128, 3, 128], f, tag="ob")
            if rf % 2 == 0:
                nc.scalar.copy(ob[:], psf[:, :N])
            else:
                nc.vector.tensor_copy(ob[:], psf[:, :N])
            nc.sync.dma_start(
                out_s[rf * N:(rf + 1) * N, :].rearrange("(c p) d -> p c d", p=128),
                ob[:],
            )

        ogs = {}
        for t in range(UNITS + STAGGER + 3):
            if t < UNITS:
                emit_kq(t)
            m = t - STAGGER
            if 0 <= m < UNITS:
                if m % 6 == 0 and m >= 6:
                    # row rf = m//6 - 1 fully accumulated (AV of u5 emitted
                    # last iteration); normalize before this row's u0 AV
                    # overwrites pso/pss.
                    ogs[m // 6 - 1] = emit_finish_dve(m // 6 - 1)
                emit_av(m)
                if m % 6 == 2 and m >= 6:
                    rf = m // 6 - 1
                    emit_finish_pe(rf, ogs.pop(rf))
            elif m == UNITS:
                ogs[R - 1] = emit_finish_dve(R - 1)
            elif m == UNITS + 2:
                emit_finish_pe(R - 1, ogs.pop(R - 1))

        _chain(kq_chain)
        for h in range(H):
            _chain(av_chain[h])
            _chain(sm_chain[h])


_NC = None
_NC_VB = None


def _get_nc(has_vb=False):
    global _NC, _NC_VB
    if has_vb:
        if _NC_VB is None:
            _NC_VB = build_program(True)
        return _NC_VB
    if _NC is None:
        _NC = build_program(False)
    return _NC


def make_in_maps(pair, mask, ln_w, ln_b, bias_w, q_w, k_w, v_w, g_w, o_w):
    import ml_dtypes
    pair = np.asarray(pair, np.float32)
    mask = np.asarray(mask, np.float32)
    ln_w = np.asarray(ln_w, np.float32)
    ln_b = np.asarray(ln_b, np.float32)
    bias_w = np.asarray(bias_w, np.float32)
    s4 = float(D) ** -0.25

    def fold(w, scale=1.0):
        w = np.asarray(w, np.float32)
        wm = w * ln_w[None, :] * scale
        return (
            np.ascontiguousarray(wm.T).astype(ml_dtypes.bfloat16),
            np.ascontiguousarray(((w @ ln_b) * scale).reshape(C, 1)),
        )

    wq, qb = fold(q_w, s4)
    wk, kb = fold(k_w, s4)
    wv, vb = fold(v_w)
    wg, gb = fold(g_w)
    wo = np.ascontiguousarray(np.asarray(o_w, np.float32).T).astype(ml_dtypes.bfloat16)
    bwm = bias_w * ln_w[None, :]
    bwT = np.zeros((C, 32), np.float32)                    # [C, 32], heads in cols 0..4
    bwT[:, :H] = bwm.T
    bb = (bias_w @ ln_b).astype(np.float32)                # [H]
    bb128 = np.zeros((C, 1), np.float32)
    for s in range(4):
        bb128[32 * s:32 * s + 4, 0] = bb
    has_vb = bool(np.any(vb != 0.0))
    common = {
        "wq": wq, "wk": wk, "wv": wv, "wg": wg, "wo": wo,
        "bwT": bwT.astype(ml_dtypes.bfloat16),
        "qb": qb, "kb": kb,
        "gbh": np.ascontiguousarray(0.5 * gb),
        "vb": vb,
        "identb": np.eye(C, dtype=ml_dtypes.bfloat16),
        "twosb": np.full((C, D), 2.0, dtype=ml_dtypes.bfloat16),
        "bb128": bb128,
        "ident": np.eye(C, dtype=np.float32),
    }
    in_maps = []
    for c in range(NCORES):
        m = dict(common)
        m["pair_s"] = np.ascontiguousarray(
            pair[c * R:(c + 1) * R].reshape(T, C)
        )
        m["mask_s"] = np.ascontiguousarray(mask[c * R:(c + 1) * R])
        in_maps.append(m)
    return in_maps, has_vb


def kernel(**inputs) -> np.ndarray:
    in_maps, has_vb = make_in_maps(**inputs)
    nc = _get_nc(has_vb)
    res = run_bass_kernel_spmd(nc, in_maps, core_ids=list(range(NCORES)))
    shards = [res.results[c]["out_s"].reshape(R, N, C) for c in range(NCORES)]
    return np.ascontiguousarray(np.concatenate(shards, axis=0))
